# revision 40
# baseline (speedup 1.0000x reference)
"""GQA multi-head attention (B=2, S=2048, D=2048, 32 q-heads / 8 kv-heads)
on 8 Trainium2 NeuronCores.

Sharding: tensor-parallel over kv-head groups. Core c owns kv head c and its
4 query heads: Wq column-shard [2048, 256], Wk/Wv column-shard [2048, 64],
Wo row-shard [256, 2048]. Each core computes a full-shape partial output
(its heads' contribution through Wo); the host sums the 8 partials.

PE cost on TRN2 is charged per output-free-size row, so every matmul is
oriented to keep the output free dim minimal, and the dense projections run
as fp8e4 DoubleRow matmuls (0.5 cyc/row, two k-tiles per instruction) with
hi+lo error compensation: x ~ hi + lo in fp8, out = xh*Wh + xh*Wl + xl*Wh
(3 terms at 0.75x the bf16 cost; the dropped lo*lo term is ~2^-8 relative).
Weights are pre-scaled x64 on the host so their fp8 lo parts don't denormal;
the 1/64 is folded into the psum->sbuf copies.

Per-core dataflow (fp32 PSUM accumulate):
  QT  [256, B*S] = Wq_c^T @ q^T     (fp8 3-term; psum [128, 512])
  K   [keys, 64] = kT-tile^T @ Wk_c (fp8 3-term, input tile stationary:
                                     64-row charge; K^T recovered via PE
                                     transpose into ktd, dup'd to both halves)
  V   [keys, 64] = vT-tile^T @ Wv_c (same flip; lands directly in vsb layout)
  S^T [k, q] = KT-tile.T @ QT       (bf16; scores transposed: softmax axis on
                                     partitions. fp8 does not pay here: the
                                     64-deep contraction is one k-tile, so
                                     3-term DR costs 1.5x a bf16 matmul)
  expS^T = exp(S^T * 1/8)           (ACT, scale folded in; no max-sub:
                                     |scores/8| small so fp32 exp is safe.
                                     262K rows through the only exp-capable
                                     engine = the ~267us floor of this kernel)
  ctx [q, 65] = expS^T-tile.T @ V_aug  (bf16, exp tile stationary: 65-row
                                     charge vs 512 in the V-stationary
                                     orientation; ones column gives the
                                     softmax denominator in free col 64 ->
                                     normalization is a per-partition DVE
                                     tensor_scalar_mul fused into the
                                     required psum->sbuf copy)
  ctx^T via PE transpose into ctxT [dq, q]
  out_partial [B*S, 2048] = ctxT-tiles.T @ Wo_c  (bf16: ctxT hi/lo splitting
                                     on-device needs a mixed-dtype DVE
                                     subtract that miscompiles on the NEFF
                                     path, so fp8 is host-side-only tensors)

Schedule: per-slot round-robin of ~1-2.5us PE "pieces" (scores pairs vs
ctx/out-proj/next-batch-projection work) keeps the 4-deep ACT wait queue fed;
ctx lags scores by one head-step so PE's in-order stream never waits on exp;
batch-0's last out-proj tiles are deferred into batch-1's exp-bound window.

The DMA XBAR transpose (dma_start(transpose=True)) is numerically correct in
CoreSim but produces garbage/races on the compiled NEFF path here - only PE
transposes are safe (K_TRANS=pe default).
"""
from collections import deque

from contextlib import ExitStack

import numpy as np
import ml_dtypes

import jax

try:
    jax.config.update("jax_compilation_cache_dir", "/tmp/jax_bass_cache")
    jax.config.update("jax_persistent_cache_min_compile_time_secs", 1.0)
except Exception:
    pass

from jax.sharding import Mesh, PartitionSpec, NamedSharding
from jax.experimental.shard_map import shard_map

import concourse.bass as bass
import concourse.mybir as mybir
import concourse.tile as tile
from concourse import bacc, bass2jax

BF16 = mybir.dt.bfloat16
FP8 = mybir.dt.float8e4
F32 = mybir.dt.float32
AF = mybir.ActivationFunctionType

B, S, DM = 2, 2048, 2048
HKV, G, DH = 8, 4, 64
DQ = G * DH            # 256: per-core q-projection width
NC = 8
DT = DM // 128         # 16 contraction tiles
BS = B * S             # 4096
SCALE = 1.0 / 8.0      # 1/sqrt(64)

_cache = {}

import os as _os
# "pe" | "dma": the DMA XBAR route is numerically correct in CoreSim but
# races on the compiled NEFF path, so PE transposes are the default.
TRANS = _os.environ.get("K_TRANS", "pe")


def _emit(ctx, tc, qTh, qTl, kTh, kTl, vTh, vTl, wqh, wql, wkh, wkl, wvh,
          wvl, wo, out):
    nc = tc.nc

    pp = ctx.enter_context(tc.tile_pool(name="persist", bufs=1))
    wqh_sb = pp.tile([128, DT, DQ], FP8, tag="wqh")
    wql_sb = pp.tile([128, DT, DQ], FP8, tag="wql")
    wkh_sb = pp.tile([128, DT, DH], FP8, tag="wkh")
    wkl_sb = pp.tile([128, DT, DH], FP8, tag="wkl")
    wvh_sb = pp.tile([128, DT, DH], FP8, tag="wvh")
    wvl_sb = pp.tile([128, DT, DH], FP8, tag="wvl")
    wo_sb = pp.tile([128, 2, DM], BF16, tag="wo")
    qtp = pp.tile([128, 2, BS], BF16, tag="qtp")    # QT pairs [p, hp, b*S+s]
    ktd = pp.tile([128, BS], BF16, tag="ktd")       # KT duplicated both halves
    vsb = pp.tile([128, BS // 128, DH + 1], BF16, tag="vsb")  # V + ones col
    ctxT = pp.tile([128, 2, BS], BF16, tag="ctxT")  # normalized ctx^T pairs

    if TRANS in ("pe", "kdma"):
        ident = pp.tile([128, 128], BF16, tag="ident")
        from concourse.masks import make_identity
        make_identity(nc, ident[:])

    # Weight DMA issue order matters: DMA sem-waits hold the SP sequencer,
    # and the lead-in is gated by K/V arrival. wk/wv/wq first (tiny), K/V
    # chunks next, wo deferred past the first q chunk.
    for w_sb, w in ((wkh_sb, wkh), (wkl_sb, wkl), (wvh_sb, wvh),
                    (wvl_sb, wvl)):
        nc.sync.dma_start(w_sb[:], w.rearrange("(dt p) m -> p dt m", p=128))
    nc.gpsimd.memset(vsb[:, :, DH], 1.0)

    stage = ctx.enter_context(tc.tile_pool(name="stage", bufs=2))
    expp = ctx.enter_context(tc.tile_pool(name="expp", bufs=2))
    smal = ctx.enter_context(tc.tile_pool(name="small", bufs=2))
    outp = ctx.enter_context(tc.tile_pool(name="outp", bufs=2))
    psum = ctx.enter_context(tc.tile_pool(name="psum", bufs=1, space="PSUM"))

    DIV = mybir.AluOpType.divide

    # ---- work-piece pump: ~1-2.5us PE pieces round-robined between the
    # scores pieces of each slot so the 4-deep ACT wait queue never starves.
    work = deque()

    def pump_one():
        while work:
            try:
                next(work[0])
                return True
            except StopIteration:
                work.popleft()
        return False

    def run_gen(g):
        for _ in g:
            pass

    def advance(g):
        return lambda: next(g, None)

    # ---- Phase A emitters -------------------------------------------------
    def a_kv(b, qc):
        """K/V projections, input-tile-stationary: psum [128 keys, 64].
        DMA issue is eager (at call time); compute comes as gen pieces."""
        bo, so = b * S, qc * 512
        chs = {}
        for nm, src_t in (("kh", kTh), ("kl", kTl), ("vh", vTh), ("vl", vTl)):
            ch = stage.tile([128, DT, 512], FP8, tag="kv_st", bufs=6,
                            name=f"{nm}ch_{b}_{qc}")
            nc.sync.dma_start(
                ch[:],
                src_t[b].rearrange("(dt p) s -> p dt s", p=128)[:, :,
                                                               so:so + 512])
            chs[nm] = ch

        def gen():
            kv = psum.tile([128, 4, 2, DH], F32, tag="pa", bufs=2,
                           name=f"kv_{b}_{qc}")
            kterms = ((chs["kh"], wkh_sb), (chs["kh"], wkl_sb),
                      (chs["kl"], wkh_sb))
            vterms = ((chs["vh"], wvh_sb), (chs["vh"], wvl_sb),
                      (chs["vl"], wvh_sb))
            for ki in range(4):
                for col, terms in ((0, kterms), (1, vterms)):
                    n = 0
                    for ch, w_sb in terms:
                        for t in range(DT // 2):
                            nc.tensor.matmul(
                                kv[:, ki, col, :],
                                ch[:, 2 * t:2 * t + 2,
                                   ki * 128:(ki + 1) * 128],
                                w_sb[:, 2 * t:2 * t + 2, :],
                                start=(n == 0), stop=(n == 3 * DT // 2 - 1),
                                perf_mode=mybir.MatmulPerfMode.DoubleRow)
                            n += 1
                yield
            nc.vector.tensor_scalar_mul(
                vsb[:, b * 16 + qc * 4:b * 16 + qc * 4 + 4, 0:DH],
                kv[:, :, 1, :], 1.0 / 64.0)
            ktmp = smal.tile([128, 4, DH], BF16, tag="ktmp", bufs=2,
                             name=f"ktmp_{b}_{qc}")
            nc.vector.tensor_scalar_mul(ktmp[:], kv[:, :, 0, :], 1.0 / 64.0)
            yield
            if TRANS in ("dma", "kdma"):
                for pr in range(2):
                    src = ktmp[:, 2 * pr:2 * pr + 2, :]
                    for half in range(2):
                        dst = ktd[DH * half:DH * half + DH,
                                  bo + so + pr * 256:bo + so + (pr + 1) * 256]
                        nc.sync.dma_start(
                            dst.rearrange("p (t k) -> p t k", t=2), src,
                            transpose=True)
            else:
                for ki in range(4):
                    koff = bo + so + ki * 128
                    ktp = psum.tile([128, 128], BF16, tag="pc", bufs=2,
                                    name=f"ktp_{b}_{qc}_{ki}")
                    for half in range(2):
                        nc.tensor.transpose(
                            ktp[DH * half:DH * half + DH, :], ktmp[:, ki, :],
                            ident[:], tile_position=(0, DH * half))
                    nc.vector.tensor_copy(ktd[:, koff:koff + 128], ktp[:])
            yield

        return gen()

    def a_q_dma(b, qc):
        bo, so = b * S, qc * 512
        qh_ch = stage.tile([128, DT, 512], FP8, tag="q_st", bufs=3,
                           name=f"qhch_{b}_{qc}")
        nc.sync.dma_start(
            qh_ch[:],
            qTh[b].rearrange("(dt p) s -> p dt s", p=128)[:, :, so:so + 512])
        ql_ch = stage.tile([128, DT, 512], FP8, tag="q_st", bufs=3,
                           name=f"qlch_{b}_{qc}")
        nc.sync.dma_start(
            ql_ch[:],
            qTl[b].rearrange("(dt p) s -> p dt s", p=128)[:, :, so:so + 512])
        return (qh_ch, ql_ch)

    def a_q_proj(b, qc, q_ch):
        bo, so = b * S, qc * 512
        qh_ch, ql_ch = q_ch
        terms = [(wqh_sb, qh_ch), (wqh_sb, ql_ch), (wql_sb, qh_ch)]
        for m in range(2):
            pq = psum.tile([128, 512], F32, tag="pa", bufs=2,
                           name=f"pq_{b}_{qc}_{m}")
            n = 0
            for w_sb, qch in terms:
                for t in range(DT // 2):
                    nc.tensor.matmul(
                        pq[:], w_sb[:, 2 * t:2 * t + 2, m * 128:(m + 1) * 128],
                        qch[:, 2 * t:2 * t + 2, :],
                        start=(n == 0), stop=(n == 3 * DT // 2 - 1),
                        perf_mode=mybir.MatmulPerfMode.DoubleRow)
                    n += 1
            nc.vector.tensor_scalar_mul(
                qtp[:, m, bo + so:bo + so + 512], pq[:], 1.0 / 64.0)
            yield

    # ---- Phase C piece ----------------------------------------------------
    def c_st(b, qc, qt, ptag="pa"):
        st = b * 16 + qc * 4 + qt
        ost = outp.tile([128, DM], BF16, tag="ost", bufs=3, name=f"ost_{st}")
        for ch in range(4):
            po = psum.tile([128, 512], F32, tag=ptag, bufs=2,
                           name=f"po_{st}_{ch}")
            for i in range(2):
                nc.tensor.matmul(
                    po[:], ctxT[:, i, st * 128:(st + 1) * 128],
                    wo_sb[:, i, ch * 512:(ch + 1) * 512],
                    start=(i == 0), stop=(i == 1))
            nc.vector.tensor_copy(ost[:, ch * 512:(ch + 1) * 512], po[:])
        nc.sync.dma_start(out[st * 128:(st + 1) * 128, :], ost[:])

    # ---- Phase B emitters -------------------------------------------------
    cn_map = {}
    c_backlog = []

    def ctx_gen(b, qc, h, ex, defer_c=False):
        """ctx [q, 65] with exp tile stationary; normalization fused into the
        required psum->sbuf copy; per-qt transposes + out-proj pieces."""
        i, j = h // 2, h % 2
        if j == 0:
            cn_map[(b, qc, i)] = [
                smal.tile([128, 2, DH], BF16, tag="cn", bufs=8,
                          name=f"cn_{b}_{qc}_{i}_{qt}") for qt in range(4)]
        cn = cn_map[(b, qc, i)]
        pcx = psum.tile([128, 4, DH + 1], F32, tag="pc", bufs=2,
                        name=f"pcx_{b}_{qc}_{h}")
        for qt2 in range(2):
            for qt in (2 * qt2, 2 * qt2 + 1):
                for kt in range(DT):
                    nc.tensor.matmul(
                        pcx[:, qt, :], ex[:, kt, qt * 128:(qt + 1) * 128],
                        vsb[:, b * 16 + kt, :],
                        start=(kt == 0), stop=(kt == DT - 1))
            yield
        rr = smal.tile([128, 4], F32, tag="rr", bufs=3, name=f"rr_{b}_{qc}_{h}")
        nc.vector.reciprocal(rr[:], pcx[:, :, DH])
        for qt in range(4):
            nc.vector.tensor_scalar_mul(
                cn[qt][:, j, :], pcx[:, qt, 0:DH], rr[:, qt:qt + 1])
        yield
        if j == 1:
            qoff = b * S + qc * 512
            for qt in range(4):
                dst = ctxT[:, i, qoff + qt * 128:qoff + (qt + 1) * 128]
                if TRANS == "dma":
                    nc.sync.dma_start(dst, cn[qt][:], transpose=True)
                elif True:
                    ctp = psum.tile([128, 128], BF16, tag="pc", bufs=2,
                                    name=f"ctp_{b}_{qc}_{i}_{qt}")
                    nc.tensor.transpose(ctp[:], cn[qt][:], ident[:])
                    nc.vector.tensor_copy(dst, ctp[:])
                if i == 1:
                    if defer_c:
                        c_backlog.append((b, qc, qt))
                    else:
                        c_st(b, qc, qt, "pc" if b == 0 else "pa")
                yield
            del cn_map[(b, qc, i)]

    def scores_slot(b, qc, h, actions):
        """Emit the 8 score/exp pieces of slot (qc, h), interleaving one
        action (forced work or deque pump) after each piece."""
        m, j = h // 2, h % 2
        bo = b * S
        qoff = bo + qc * 512
        ex = expp.tile([128, DT, 512], BF16, tag="exp", bufs=3,
                       name=f"ex_{b}_{qc}_{h}")
        for kt2 in range(DT // 2):
            pss = psum.tile([128, 2, 512], F32, tag="sc", bufs=2,
                            name=f"pss_{b}_{qc}_{h}_{kt2}")
            for t in range(2):
                koff = bo + (2 * kt2 + t) * 128
                nc.tensor.matmul(
                    pss[:, t, :], ktd[j * DH:(j + 1) * DH, koff:koff + 128],
                    qtp[j * DH:(j + 1) * DH, m, qoff:qoff + 512])
            nc.scalar.activation(
                ex[:, 2 * kt2:2 * kt2 + 2, :], pss[:], AF.Exp, scale=SCALE)
            # ctx lags two slots, so its exps are long done - pump freely.
            if kt2 < len(actions):
                actions[kt2]()
            else:
                pump_one()
        for act in actions[DT // 2:]:
            act()
        pump_one()
        pump_one()
        return ex

    def last_slot(b, qc):
        """Final slot (b=1, qc=3, h=3): fuse ctx into the scores stream at
        kt granularity (one exp pair behind) so the post-exp tail is just
        the last pair's ctx + normalize + transpose + out-proj."""
        m, j, i = 1, 1, 1
        bo = b * S
        qoff = bo + qc * 512
        ex = expp.tile([128, DT, 512], BF16, tag="exp", bufs=3,
                       name=f"ex_{b}_{qc}_3f")
        pcx = psum.tile([128, 4, DH + 1], F32, tag="pa", bufs=2,
                        name=f"pcxl_{b}_{qc}")

        def chase(qt, kt):
            # one PSUM accumulation group open at a time per bank: only qt
            # can chase; the other q-tiles run after the exp stream ends.
            nc.tensor.matmul(
                pcx[:, qt, :], ex[:, kt, qt * 128:(qt + 1) * 128],
                vsb[:, b * 16 + kt, :],
                start=(kt == 0), stop=(kt == DT - 1))

        for kt2 in range(DT // 2):
            pss = psum.tile([128, 2, 512], F32, tag="sc", bufs=2,
                            name=f"pss_{b}_{qc}_3_{kt2}")
            for t in range(2):
                koff = bo + (2 * kt2 + t) * 128
                nc.tensor.matmul(
                    pss[:, t, :], ktd[j * DH:(j + 1) * DH, koff:koff + 128],
                    qtp[j * DH:(j + 1) * DH, m, qoff:qoff + 512])
            nc.scalar.activation(
                ex[:, 2 * kt2:2 * kt2 + 2, :], pss[:], AF.Exp, scale=SCALE)
            if kt2 >= 1:
                chase(0, 2 * kt2 - 2)
                chase(0, 2 * kt2 - 1)
            if kt2 >= 2:
                pump_one()
        while pump_one():   # finish ctx(qc, 2): writes cn[:, 0, :]
            pass
        chase(0, DT - 2)
        chase(0, DT - 1)
        for qt in range(1, 4):
            for kt in range(DT):
                chase(qt, kt)
        cn = cn_map[(b, qc, i)]
        rr = smal.tile([128, 4], F32, tag="rr", bufs=3, name=f"rrl_{b}_{qc}")
        nc.vector.reciprocal(rr[:], pcx[:, :, DH])
        for qt in range(4):
            nc.vector.tensor_scalar_mul(
                cn[qt][:, 1, :], pcx[:, qt, 0:DH], rr[:, qt:qt + 1])
        for qt in range(4):
            dst = ctxT[:, i, qoff + qt * 128:qoff + (qt + 1) * 128]
            ctp = psum.tile([128, 128], BF16, tag="pc", bufs=2,
                            name=f"ctpl_{b}_{qc}_{qt}")
            nc.tensor.transpose(ctp[:], cn[qt][:], ident[:])
            nc.vector.tensor_copy(dst, ctp[:])
            c_st(b, qc, qt, "pa")
        del cn_map[(b, qc, i)]

    # ---- Schedule ---------------------------------------------------------
    # Lead-in: K/V chunks stream while projections chase; first q chunk's
    # DMA is slotted before the last K/V pair so q-proj overlaps the tail.
    g00 = a_kv(0, 0)
    nc.sync.dma_start(wqh_sb[:], wqh.rearrange("(dt p) m -> p dt m", p=128))
    nc.sync.dma_start(wql_sb[:], wql.rearrange("(dt p) m -> p dt m", p=128))
    run_gen(g00)
    run_gen(a_kv(0, 1))
    g2 = a_kv(0, 2)
    q_ch0 = a_q_dma(0, 0)          # q0 DMA ahead of the last K/V pair: its
    run_gen(g2)                    # projection overlaps kv-proj(3)'s wait
    g_last = a_kv(0, 3)
    run_gen(a_q_proj(0, 0, q_ch0))
    run_gen(g_last)
    nc.sync.dma_start(wo_sb[:], wo.rearrange("(i p) d -> p i d", p=128))

    q_chunks = {}
    q_projs = {}

    for b in range(B):
        pend = deque()
        for qc in range(4):
            for h in range(4):
                if b == 1 and qc == 3 and h == 3:
                    while pend:
                        prev = pend.popleft()
                        work.append(ctx_gen(b, *prev, defer_c=False))
                    last_slot(b, qc)
                    continue
                actions = []
                if h == 0 and qc < 3:
                    # q chunk (b, qc+1): DMA now, proj pieces forced next slot
                    q_chunks[(b, qc + 1)] = a_q_dma(b, qc + 1)
                    g = a_q_proj(b, qc + 1, q_chunks[(b, qc + 1)])
                    q_projs[(b, qc + 1)] = g
                    actions += [advance(g), advance(g)]
                if b == 0 and h == 1:
                    g = a_kv(1, qc)
                    actions += [advance(g)] * 3
                    work.append(g)
                if b == 0 and qc == 3 and h == 2:
                    q_chunks[(1, 0)] = a_q_dma(1, 0)
                if b == 1 and c_backlog:
                    bb, bqc, bqt = c_backlog.pop(0)
                    actions.append(
                        lambda bb=bb, bqc=bqc, bqt=bqt: c_st(bb, bqc, bqt))
                if len(pend) >= 2:
                    prev = pend.popleft()
                    work.append(ctx_gen(b, *prev,
                                        defer_c=(b == 0 and prev[0] >= 2)))
                ex = scores_slot(b, qc, h, actions)
                pend.append((qc, h, ex))
        while pend:
            prev = pend.popleft()
            work.append(ctx_gen(b, *prev, defer_c=(b == 0)))
        if b == 0:
            run_gen(a_q_proj(1, 0, q_chunks[(1, 0)]))
        while pump_one():
            pass


def _build():
    nc = bacc.Bacc("TRN2", target_bir_lowering=False, debug=False, num_devices=NC)
    qTh = nc.dram_tensor("qTh", [B, DM, S], FP8, kind="ExternalInput")
    qTl = nc.dram_tensor("qTl", [B, DM, S], FP8, kind="ExternalInput")
    kTh = nc.dram_tensor("kTh", [B, DM, S], FP8, kind="ExternalInput")
    kTl = nc.dram_tensor("kTl", [B, DM, S], FP8, kind="ExternalInput")
    vTh = nc.dram_tensor("vTh", [B, DM, S], FP8, kind="ExternalInput")
    vTl = nc.dram_tensor("vTl", [B, DM, S], FP8, kind="ExternalInput")
    wqh = nc.dram_tensor("wqh", [DM, DQ], FP8, kind="ExternalInput")
    wql = nc.dram_tensor("wql", [DM, DQ], FP8, kind="ExternalInput")
    wkh = nc.dram_tensor("wkh", [DM, DH], FP8, kind="ExternalInput")
    wkl = nc.dram_tensor("wkl", [DM, DH], FP8, kind="ExternalInput")
    wvh = nc.dram_tensor("wvh", [DM, DH], FP8, kind="ExternalInput")
    wvl = nc.dram_tensor("wvl", [DM, DH], FP8, kind="ExternalInput")
    wo = nc.dram_tensor("wo", [DQ, DM], BF16, kind="ExternalInput")
    out = nc.dram_tensor("out", [BS, DM], BF16, kind="ExternalOutput")
    with tile.TileContext(nc) as tc:
        with ExitStack() as ctx:
            _emit(ctx, tc, qTh.ap(), qTl.ap(), kTh.ap(), kTl.ap(), vTh.ap(),
                  vTl.ap(), wqh.ap(), wql.ap(), wkh.ap(), wkl.ap(), wvh.ap(),
                  wvl.ap(), wo.ap(), out.ap())
    nc.compile()
    return nc


def _make_runner(nc, n_cores=NC):
    """Build the sharded jit callable once; reuse across kernel() calls."""
    bass2jax.install_neuronx_cc_hook()
    partition_name = nc.partition_id_tensor.name if nc.partition_id_tensor else None
    in_names, out_names, out_avals, zero_outs = [], [], [], []
    for alloc in nc.m.functions[0].allocations:
        if not isinstance(alloc, mybir.MemoryLocationSet):
            continue
        name = alloc.memorylocations[0].name
        if alloc.kind == "ExternalInput":
            if name != partition_name:
                in_names.append(name)
        elif alloc.kind == "ExternalOutput":
            out_names.append(name)
            shape = tuple(alloc.tensor_shape)
            dtype = mybir.dt.np(alloc.dtype)
            out_avals.append(jax.core.ShapedArray(shape, dtype))
            zero_outs.append(np.zeros(shape, dtype))
    n_params = len(in_names)
    n_outs = len(out_avals)
    in_names_all = in_names + out_names
    if partition_name is not None:
        in_names_all.append(partition_name)
    donate = tuple(range(n_params, n_params + n_outs))

    def _body(*args):
        operands = list(args)
        if partition_name is not None:
            operands.append(bass2jax.partition_id_tensor())
        outs = bass2jax._bass_exec_p.bind(
            *operands,
            out_avals=tuple(out_avals),
            in_names=tuple(in_names_all),
            out_names=tuple(out_names),
            lowering_input_output_aliases=(),
            sim_require_finite=True,
            sim_require_nnan=True,
            nc=nc,
        )
        return tuple(outs)

    devices = jax.devices()[:n_cores]
    mesh = Mesh(np.asarray(devices), ("core",))
    in_specs = (PartitionSpec("core"),) * (n_params + n_outs)
    out_specs = (PartitionSpec("core"),) * len(out_names)
    sharded = jax.jit(
        shard_map(_body, mesh=mesh, in_specs=in_specs, out_specs=out_specs,
                  check_rep=False),
        donate_argnums=donate, keep_unused=True)
    sh = NamedSharding(mesh, PartitionSpec("core"))
    return sharded, in_names, out_names, zero_outs, sh


def _run(in_maps):
    if "nc" not in _cache:
        _cache["nc"] = _build()
    if "runner" not in _cache:
        _cache["runner"] = _make_runner(_cache["nc"])
    sharded, in_names, out_names, zero_outs, sh = _cache["runner"]
    n = NC
    concat_in = [
        jax.device_put(
            np.concatenate([np.asarray(in_maps[c][nm]) for c in range(n)], 0), sh)
        for nm in in_names
    ]
    zeros = [
        jax.device_put(np.zeros((n * z.shape[0], *z.shape[1:]), z.dtype), sh)
        for z in zero_outs
    ]
    outs = sharded(*concat_in, *zeros)
    i = out_names.index("out")
    arr = np.asarray(outs[i])           # [NC*BS, DM]
    return arr.reshape(n, BS, DM)


def kernel(q, k, v, Wq, Wk, Wv, Wo):
    q = np.asarray(q, dtype=np.float32)
    k = np.asarray(k, dtype=np.float32)
    v = np.asarray(v, dtype=np.float32)
    bf = ml_dtypes.bfloat16
    f8 = ml_dtypes.float8_e4m3
    qT32 = np.ascontiguousarray(q.transpose(0, 2, 1))
    qThi = qT32.astype(f8)
    qTlo = (qT32 - qThi.astype(np.float32)).astype(f8)
    kT32 = np.ascontiguousarray(k.transpose(0, 2, 1))
    kThi = kT32.astype(f8)
    kTlo = (kT32 - kThi.astype(np.float32)).astype(f8)
    vT32 = np.ascontiguousarray(v.transpose(0, 2, 1))
    vThi = vT32.astype(f8)
    vTlo = (vT32 - vThi.astype(np.float32)).astype(f8)
    Wq64 = np.asarray(Wq, dtype=np.float32) * 64.0
    Wqhi = Wq64.astype(f8)
    Wqlo = (Wq64 - Wqhi.astype(np.float32)).astype(f8)
    Wk64 = np.asarray(Wk, dtype=np.float32) * 64.0
    Wkhi = Wk64.astype(f8)
    Wklo = (Wk64 - Wkhi.astype(np.float32)).astype(f8)
    Wv64 = np.asarray(Wv, dtype=np.float32) * 64.0
    Wvhi = Wv64.astype(f8)
    Wvlo = (Wv64 - Wvhi.astype(np.float32)).astype(f8)
    Wob = np.asarray(Wo, dtype=np.float32).astype(bf)

    in_maps = []
    for c in range(NC):
        in_maps.append({
            "qTh": qThi, "qTl": qTlo, "kTh": kThi, "kTl": kTlo,
            "vTh": vThi, "vTl": vTlo,
            "wqh": np.ascontiguousarray(Wqhi[:, c * DQ:(c + 1) * DQ]),
            "wql": np.ascontiguousarray(Wqlo[:, c * DQ:(c + 1) * DQ]),
            "wkh": np.ascontiguousarray(Wkhi[:, c * DH:(c + 1) * DH]),
            "wkl": np.ascontiguousarray(Wklo[:, c * DH:(c + 1) * DH]),
            "wvh": np.ascontiguousarray(Wvhi[:, c * DH:(c + 1) * DH]),
            "wvl": np.ascontiguousarray(Wvlo[:, c * DH:(c + 1) * DH]),
            "wo": np.ascontiguousarray(Wob[c * DQ:(c + 1) * DQ, :]),
        })
    partials = _run(in_maps)
    out = partials.astype(np.float32, copy=False).sum(axis=0)
    return out.reshape(B, S, DM)

